# revision 1
# baseline (speedup 1.0000x reference)
"""Cross-attention kernel for Trainium2 (Bass/Tile), 8-core SPMD.

Computes, per batch b:
    S = enc_b @ dec_b.T            # [T_enc, T_dec]
    A = softmax(S, axis=T_enc)
    C = A.T @ enc_b                # [T_dec, D]
which equals standard attention with Q=dec, K=V=enc (softmax over keys).

Sharding: 8 cores = 4 batches x 2 query-halves. Each core handles
Q = dec[b, th*2048:(th+1)*2048] against K=V=enc[b] (4096 keys).

Per-core algorithm (flash-style over 2 key-halves to fit SBUF):
  for half h in {0,1} (2048 keys each):
    load enc half native [e,d]; build encT [d,e] via PE transposes
    for each 128-query block:
      S = decT.T @ encT (fp32r matmuls, PSUM)
      m = rowmax(S); P = exp(S-m) with rowsum l (ACT, split in 2 halves)
      O_h = P @ enc_half (PE-transpose P tiles; fp32r matmuls)
      h==0: stash O_0 in DRAM scratch, m_0/l_0 in SBUF
      h==1: C = (exp(m0-m)*O0 + exp(m1-m)*O1) / (e^{m0-m} l0 + e^{m1-m} l1)

Engine-balance notes: PT/encT PSUM evacuations alternate DVE/ACT so the
vector engine stays off the PE critical path; the dec load+transpose chain
for block qb+1 is emitted before block qb's softmax so PE fills the exp gap.
"""

import numpy as np

import concourse.bass as bass
import concourse.mybir as mybir
import concourse.tile as tile
from concourse import bacc
from concourse.bass_utils import run_bass_kernel_spmd
from concourse.masks import make_identity

P = 128
E = 4096          # keys (T_enc)
D = 1024
TQ = 2048         # queries per core
NHALF = 2
EH = E // NHALF   # 2048
NET = EH // P     # 16 e-subtiles per half
NDC = D // P      # 8 d-chunks
NQB = TQ // P     # 16 query blocks
MM1_NT = EH // 512  # 4
MM2_NT = D // 512   # 2

F32 = mybir.dt.float32
F32R = mybir.dt.float32r
AX = mybir.AxisListType.X
EXP = mybir.ActivationFunctionType.Exp
MAX = mybir.AluOpType.max
MIN = mybir.AluOpType.min
MULT = mybir.AluOpType.mult
ADD = mybir.AluOpType.add


def _r(ap):
    """fp32 -> fp32r view (1 cycle/row matmul at N>=256 vs 4 for fp32)."""
    return ap.bitcast(F32R)


def build_nc():
    nc = bacc.Bacc(None, target_bir_lowering=False)
    enc = nc.dram_tensor("enc", [E, D], F32, kind="ExternalInput")
    dec = nc.dram_tensor("dec", [TQ, D], F32, kind="ExternalInput")
    out = nc.dram_tensor("out", [TQ, D], F32, kind="ExternalOutput")

    with tile.TileContext(nc) as tc:
        with (
            tc.tile_pool(name="const", bufs=1) as const_pool,
            tc.tile_pool(name="encnat", bufs=1) as enc_nat_pool,
            tc.tile_pool(name="encT", bufs=1) as encT_pool,
            tc.tile_pool(name="decp", bufs=3) as dec_pool,
            tc.tile_pool(name="decT", bufs=3) as decT_pool,
            tc.tile_pool(name="pp", bufs=2) as p_pool,
            tc.tile_pool(name="pt", bufs=5) as pt_pool,
            tc.tile_pool(name="stat", bufs=1) as stat_pool,
            tc.tile_pool(name="wstat", bufs=8) as wstat_pool,
            tc.tile_pool(name="osb", bufs=2) as osb_pool,
            tc.tile_pool(name="orb", bufs=2) as orb_pool,
            tc.tile_pool(name="tp_ps", bufs=2, space="PSUM") as tp_ps_pool,
            tc.tile_pool(name="sA_ps", bufs=2, space="PSUM") as sA_ps_pool,
            tc.tile_pool(name="sB_ps", bufs=1, space="PSUM") as sB_ps_pool,
            tc.tile_pool(name="o_ps", bufs=1, space="PSUM") as o_ps_pool,
            tc.tile_pool(name="dram", bufs=1, space="DRAM") as dram_pool,
        ):
            ident0 = const_pool.tile([P, P], F32, tag="ident0")
            make_identity(nc, ident0[:])
            # Re-write via DVE so transposes depend on a DVE write (keeps the
            # first is_transpose matmul off the gpsimd sem).
            ident = const_pool.tile([P, P], F32, tag="ident")
            nc.vector.tensor_copy(out=_r(ident[:]), in_=ident0[:])

            # -max and rowsum from half 0, one column per query block
            m0_all = stat_pool.tile([P, NQB], F32, tag="m0")
            l0_all = stat_pool.tile([P, NQB], F32, tag="l0")
            o0_scratch = dram_pool.tile([NQB * P, D], F32, tag="o0scratch")

            def load_dec_block(qb):
                """DMA dec block qb and PE-transpose it into a decT tile."""
                dec_nat = dec_pool.tile([P, D], F32, tag="dec")
                nc.sync.dma_start(
                    out=_r(dec_nat[:]), in_=_r(dec[qb * P : (qb + 1) * P, :])
                )
                decT = decT_pool.tile([P, NDC, P], F32, tag="decT")
                for dq in range(NDC // 4):
                    tq = tp_ps_pool.tile([P, 4, P], F32, tag="tp")
                    for j in range(4):
                        dc = dq * 4 + j
                        nc.tensor.transpose(
                            _r(tq[:, j, :]),
                            _r(dec_nat[:, dc * P : (dc + 1) * P]),
                            _r(ident[:]),
                        )
                    nc.vector.tensor_copy(
                        out=_r(decT[:, dq * 4 : dq * 4 + 4, :]), in_=tq[:]
                    )
                return decT

            for h in range(NHALF):
                e0 = h * EH
                enc_nat = enc_nat_pool.tile([P, NET, D], F32, tag="encnat")
                for et in range(NET):
                    nc.sync.dma_start(
                        out=_r(enc_nat[:, et, :]),
                        in_=_r(enc[e0 + et * P : e0 + (et + 1) * P, :]),
                    )
                encT = encT_pool.tile([P, NDC, EH], F32, tag="encT")
                for et in range(NET):
                    for dq in range(NDC // 4):
                        tq = tp_ps_pool.tile([P, 4, P], F32, tag="tp")
                        for j in range(4):
                            dc = dq * 4 + j
                            nc.tensor.transpose(
                                _r(tq[:, j, :]),
                                _r(enc_nat[:, et, dc * P : (dc + 1) * P]),
                                _r(ident[:]),
                            )
                        nc.vector.tensor_copy(
                            out=_r(
                                encT[:, dq * 4 : dq * 4 + 4, et * P : (et + 1) * P]
                            ),
                            in_=tq[:],
                        )

                def emit_scores(qb, decT):
                    """mm1 + softmax for block qb; returns state for mm2/evac."""
                    o0_rb = None
                    if h == 1:
                        o0_rb = orb_pool.tile([P, D], F32, tag="orb")
                        nc.sync.dma_start(
                            out=o0_rb[:], in_=o0_scratch[qb * P : (qb + 1) * P, :]
                        )
                    s_psA = sA_ps_pool.tile([P, 512], F32, tag="sA")
                    s_psB = sB_ps_pool.tile([P, EH - 512], F32, tag="sB")
                    pmax = wstat_pool.tile([P, MM1_NT], F32, tag="pmax")
                    for nt in range(MM1_NT):
                        if nt == 0:
                            s_dst = s_psA[:, 0:512]
                        else:
                            s_dst = s_psB[:, (nt - 1) * 512 : nt * 512]
                        for dc in range(NDC):
                            nc.tensor.matmul(
                                s_dst,
                                _r(decT[:, dc, :]),
                                _r(encT[:, dc, nt * 512 : (nt + 1) * 512]),
                                start=(dc == 0),
                                stop=(dc == NDC - 1),
                            )
                        nc.vector.reduce_max(
                            out=pmax[:, nt : nt + 1], in_=s_dst, axis=AX, op=MAX,
                        )
                    if h == 0:
                        negm = m0_all[:, qb : qb + 1]
                        lsum = l0_all[:, qb : qb + 1]
                    else:
                        negm_t = wstat_pool.tile([P, 1], F32, tag="negm")
                        lsum_t = wstat_pool.tile([P, 1], F32, tag="lsum")
                        negm = negm_t[:]
                        lsum = lsum_t[:]
                    nc.vector.reduce_max(
                        out=negm, in_=pmax[:], axis=AX, op=MAX, negate=True,
                    )
                    p_sb = p_pool.tile([P, EH], F32, tag="p")
                    lsumA = wstat_pool.tile([P, 1], F32, tag="lsumA")
                    lsumB = wstat_pool.tile([P, 1], F32, tag="lsumB")
                    nc.scalar.activation(
                        out=_r(p_sb[:, :512]), in_=s_psA[:], func=EXP,
                        bias=negm, scale=1.0, accum_out=lsumA[:],
                    )
                    nc.scalar.activation(
                        out=_r(p_sb[:, 512:]), in_=s_psB[:], func=EXP,
                        bias=negm, scale=1.0, accum_out=lsumB[:],
                    )
                    nc.vector.tensor_add(out=lsum, in0=lsumA[:], in1=lsumB[:])
                    return p_sb, negm, lsum, o0_rb

                decT = load_dec_block(0)
                state = emit_scores(0, decT)
                for qb in range(NQB):
                    p_sb, negm, lsum, o0_rb = state
                    # software pipeline: next block's dec prep + scores +
                    # softmax are emitted BEFORE this block's mm2, so PE fills
                    # the exp gap with real matmul work
                    if qb + 1 < NQB:
                        decT = load_dec_block(qb + 1)
                        state = emit_scores(qb + 1, decT)

                    # ---- O_h = P @ enc_half for block qb ----
                    if h == 1:
                        # combine scalars first so fin work can chase each
                        # on-pass immediately
                        negm0 = m0_all[:, qb : qb + 1]
                        l0 = l0_all[:, qb : qb + 1]
                        # negm holds -m1; negmm = -max(m0,m1) = min(negm0, negm1)
                        negmm = wstat_pool.tile([P, 1], F32, tag="negmm")
                        nc.vector.tensor_tensor(
                            out=negmm[:], in0=negm0, in1=negm, op=MIN,
                        )
                        # a_h = exp(m_h - m) = exp(negmm - negm_h)
                        d0 = wstat_pool.tile([P, 1], F32, tag="d0")
                        d1 = wstat_pool.tile([P, 1], F32, tag="d1")
                        nc.vector.tensor_sub(out=d0[:], in0=negmm[:], in1=negm0)
                        nc.vector.tensor_sub(out=d1[:], in0=negmm[:], in1=negm)
                        a0 = wstat_pool.tile([P, 1], F32, tag="a0")
                        a1 = wstat_pool.tile([P, 1], F32, tag="a1")
                        nc.scalar.activation(out=a0[:], in_=d0[:], func=EXP)
                        nc.scalar.activation(out=a1[:], in_=d1[:], func=EXP)
                        # l = a0*l0 + a1*l1; s_h = a_h / l
                        t0 = wstat_pool.tile([P, 1], F32, tag="t0")
                        nc.vector.tensor_mul(out=t0[:], in0=a0[:], in1=l0)
                        t1 = wstat_pool.tile([P, 1], F32, tag="t1")
                        nc.vector.tensor_mul(out=t1[:], in0=a1[:], in1=lsum)
                        lfull = wstat_pool.tile([P, 1], F32, tag="lfull")
                        nc.vector.tensor_add(out=lfull[:], in0=t0[:], in1=t1[:])
                        linv = wstat_pool.tile([P, 1], F32, tag="linv")
                        nc.vector.reciprocal(out=linv[:], in_=lfull[:])
                        s0 = wstat_pool.tile([P, 1], F32, tag="s0")
                        s1 = wstat_pool.tile([P, 1], F32, tag="s1")
                        nc.vector.tensor_mul(out=s0[:], in0=a0[:], in1=linv[:])
                        nc.vector.tensor_mul(out=s1[:], in0=a1[:], in1=linv[:])
                        fin = osb_pool.tile([P, D], F32, tag="fin")
                        nc.vector.tensor_scalar_mul(
                            out=fin[:], in0=o0_rb[:], scalar1=s0[:]
                        )
                    else:
                        o_sb = osb_pool.tile([P, D], F32, tag="osb")

                    ptqs = []
                    for eq in range(NET // 4):
                        tq = tp_ps_pool.tile([P, 4, P], F32, tag="tp")
                        for j in range(4):
                            et = eq * 4 + j
                            nc.tensor.transpose(
                                _r(tq[:, j, :]),
                                _r(p_sb[:, et * P : (et + 1) * P]),
                                _r(ident[:]),
                            )
                        ptq = pt_pool.tile([P, 4, P], F32, tag="pt")
                        nc.vector.tensor_copy(out=_r(ptq[:]), in_=tq[:])
                        ptqs.append(ptq)
                    for on in range(MM2_NT):
                        o_ps = o_ps_pool.tile([P, 512], F32, tag="o")
                        for et in range(NET):
                            nc.tensor.matmul(
                                o_ps[:],
                                _r(ptqs[et // 4][:, et % 4, :]),
                                _r(enc_nat[:, et, on * 512 : (on + 1) * 512]),
                                start=(et == 0),
                                stop=(et == NET - 1),
                            )
                        sl = slice(on * 512, (on + 1) * 512)
                        if h == 0:
                            nc.vector.tensor_copy(out=o_sb[:, sl], in_=o_ps[:])
                        else:
                            nc.vector.scalar_tensor_tensor(
                                out=fin[:, sl], in0=o_ps[:], scalar=s1[:],
                                in1=fin[:, sl], op0=MULT, op1=ADD,
                            )
                    if h == 0:
                        nc.sync.dma_start(
                            out=o0_scratch[qb * P : (qb + 1) * P, :], in_=o_sb[:]
                        )
                    else:
                        nc.sync.dma_start(
                            out=out[qb * P : (qb + 1) * P, :], in_=fin[:]
                        )

    # Bacc defers register allocation + wait-splitting to compile(), which
    # runs from finalize(); run_bass_via_pjrt expects a finalized module.
    nc.finalize()
    return nc


_NC_CACHE = None


def _get_nc():
    global _NC_CACHE
    if _NC_CACHE is None:
        _NC_CACHE = build_nc()
    return _NC_CACHE


def kernel(enc_output, dec_output):
    enc_np = np.ascontiguousarray(np.asarray(enc_output, dtype=np.float32))
    dec_np = np.ascontiguousarray(np.asarray(dec_output, dtype=np.float32))
    B = enc_np.shape[0]
    in_maps = []
    for core in range(8):
        b, th = core // 2, core % 2
        in_maps.append({
            "enc": np.ascontiguousarray(enc_np[b]),
            "dec": np.ascontiguousarray(dec_np[b, th * TQ : (th + 1) * TQ]),
        })
    res = run_bass_kernel_spmd(_get_nc(), in_maps, core_ids=list(range(8)))
    outp = np.empty((B, 2 * TQ, D), dtype=np.float32)
    for core in range(8):
        b, th = core // 2, core % 2
        outp[b, th * TQ : (th + 1) * TQ] = res.results[core]["out"]
    return outp

